# revision 21
# baseline (speedup 1.0000x reference)
"""MLA encoder self-attention on 8 TRN2 NeuronCores.

Sharding: data-parallel over batch (B=2) x tensor-parallel over head groups
(16 heads -> 4 groups of 4). Core c handles batch c//4, heads 4*(c%4)..+4.
Each core computes its heads' attention and a partial output projection;
the host sums the 4 head-group partials per batch.

Numerics: activations/weights are bf16 (inputs to every matmul); PSUM
accumulation is fp32; LayerNorm statistics are computed in fp32 from the
fp32 PSUM tiles. End-to-end rel err vs the f32 reference ~2e-3, well under
the 2e-2 gate.

Layout notes:
- Activations flow token-major (tokens on partitions) for LayerNorm/rope.
  Feature-major copies for the next contraction are produced by the DMA
  XBAR transpose (2-byte dtype), not the PE: stage1 emits one combined
  [128tok, 1024] tile (qcn|ckvn) per token tile and transposes it into
  cnT[:, r, :] (r 0-3 = qcn feature rows, 4-7 = ckvn); stage2 emits one
  combined [128tok, 512] tile (qf|kf) transposed into qkfT[:, j, :]
  (j 0-1 = q head-pairs, 2-3 = k head-pairs).
- kvd layout per head: [nope(32) | v(64) | ones(1)] stride 97, so v_aug
  ([v | 1]) is a contiguous 65-column slice: the AV matmul computes both
  the attention numerator and the softmax denominator (ones row) at once.
- Softmax has no max-subtraction (scores are bounded ~2, verified on host
  data) and normalization is deferred: out_aug^T rows 0..63 are scaled by
  1/Z (row 64) right before the output projection.
- Scores for two key tiles accumulate into one [128,1024] 2-bank PSUM
  tile so exp runs as half as many, twice as wide Act instructions.
"""
import numpy as np

B, S, E = 2, 2048, 1024
H, HD = 16, 64
ROPE, NOPE = 32, 32
QL, KVL = 512, 512
EPS = 1e-5
HPC = 4                # heads per core
DPC = HPC * HD         # 256
NCORES = 8
TT = S // 128          # 16 token tiles
ET = E // 128          # 8
RT = QL // 128         # 4
SB = S // 512          # 4 s-blocks
UT = S // 128          # 16 key tiles
KVW = 4 * 97           # 388: kvd width

_CACHE = {}


def _build(reps=1):
    import concourse.tile as tile
    import concourse.mybir as mybir
    from concourse import bacc

    f32 = mybir.dt.float32
    bf16 = mybir.dt.bfloat16
    AF = mybir.ActivationFunctionType
    ALU = mybir.AluOpType

    nc = bacc.Bacc("TRN2", target_bir_lowering=False, debug=False,
                   num_devices=NCORES)

    def din(name, shape, dt=bf16):
        return nc.dram_tensor(name, shape, dt, kind="ExternalInput").ap()

    xT_d = din("xT", (E, S))
    wqa_d = din("WqaT", (E, QL))
    wkva_d = din("WkvaT", (E, KVL + ROPE))
    wqb_d = din("WqbT", (QL, DPC))
    qconst_d = din("qconst", (1, DPC))
    wkvb_d = din("WkvbT", (KVL, KVW))
    kconst_d = din("kconst", (1, KVW))
    wout_d = din("WoutT", (DPC, E))
    aq_d = din("Aq", (S, DPC))
    bq_d = din("Bq", (S, DPC))
    c2k_d = din("c2k", (S, ROPE))
    s2k_d = din("s2k", (S, ROPE))
    ones1_d = din("ones1", (1, 128))
    ones4_d = din("ones4", (128, 4))
    eps_d = din("epst", (128, 1), mybir.dt.float32)
    out_d = nc.dram_tensor("out", (S, E), f32, kind="ExternalOutput").ap()

    with tile.TileContext(nc) as tc:
        import contextlib
        with contextlib.ExitStack() as top:
            consts = top.enter_context(tc.tile_pool(name="consts", bufs=1))
            ones1_t = consts.tile([1, 128], bf16, tag="ones1")
            nc.sync.dma_start(ones1_t[:], ones1_d[:])
            ones4_t = consts.tile([128, 4], bf16, tag="ones4")
            nc.sync.dma_start(ones4_t[:], ones4_d[:])
            qconst_t = consts.tile([1, DPC], bf16, tag="qconst")
            nc.sync.dma_start(qconst_t[:], qconst_d[:])
            kconst_t = consts.tile([1, KVW], bf16, tag="kconst")
            nc.sync.dma_start(kconst_t[:], kconst_d[:])
            eps_t = consts.tile([128, 1], f32, tag="epst")
            nc.sync.dma_start(eps_t[:], eps_d[:])

            acts = top.enter_context(tc.tile_pool(name="acts", bufs=1))
            wq2 = top.enter_context(tc.tile_pool(name="wq2", bufs=1))

            def body():
                # feature-major LN'd activations: r 0-3 qcn, 4-7 ckvn
                cnT = acts.tile([128, 2 * RT, S], bf16, tag="cnT")
                kpe_all = acts.tile([128, TT, ROPE], bf16, tag="kpe")

                # ---------------- stage 1: qa + kva + LN + kpe rope -------
                with contextlib.ExitStack() as st1:
                    wqa_p = st1.enter_context(tc.tile_pool(name="wqa", bufs=1))
                    wqa_t = wqa_p.tile([128, ET, QL], bf16, tag="wqa")
                    wkva_t = wqa_p.tile([128, ET, KVL + ROPE], bf16,
                                        tag="wkva")

                    pmm = st1.enter_context(
                        tc.tile_pool(name="pmm", bufs=2, space="PSUM"))
                    xp = st1.enter_context(tc.tile_pool(name="xp", bufs=3))
                    sp = st1.enter_context(tc.tile_pool(name="sp", bufs=3))
                    ck = st1.enter_context(tc.tile_pool(name="ck", bufs=1))
                    c2k_t = ck.tile([128, TT, ROPE], bf16, tag="c2k")
                    nc.gpsimd.dma_start(
                        c2k_t[:], c2k_d.rearrange("(t p) r -> p t r", p=128))
                    s2k_t = ck.tile([128, TT, ROPE], bf16, tag="s2k")
                    nc.gpsimd.dma_start(
                        s2k_t[:], s2k_d.rearrange("(t p) r -> p t r", p=128))

                    for t in range(TT):
                        ts_ = slice(t * 128, (t + 1) * 128)
                        p_qc = pmm.tile([128, QL], f32, tag="p_qc")
                        p_ka = pmm.tile([128, 272], f32, tag="p_ka")
                        p_kb = pmm.tile([128, 272], f32, tag="p_kb")
                        xt = xp.tile([128, ET, 128], bf16, tag="xt")
                        if t == 0:
                            for e in range(ET):
                                es_ = slice(e * 128, (e + 1) * 128)
                                nc.sync.dma_start(
                                    xt[:, e, :], xT_d[es_, ts_])
                                nc.scalar.dma_start(wqa_t[:, e, :],
                                                    wqa_d[es_, :])
                                nc.scalar.dma_start(wkva_t[:, e, :],
                                                    wkva_d[es_, :])
                        else:
                            nc.sync.dma_start(
                                xt[:],
                                xT_d.rearrange("(k p) s -> p k s", p=128)[:, :, ts_])
                        for e in range(ET):
                            st, sp_ = (e == 0), (e == ET - 1)
                            nc.tensor.matmul(p_qc[:], xt[:, e, :], wqa_t[:, e, :],
                                             start=st, stop=sp_)
                            nc.tensor.matmul(p_ka[:], xt[:, e, :],
                                             wkva_t[:, e, 0:272],
                                             start=st, stop=sp_)
                            nc.tensor.matmul(p_kb[:], xt[:, e, :],
                                             wkva_t[:, e, 272:544],
                                             start=st, stop=sp_)
                        # LN on qc
                        stq = sp.tile([128, 6], f32, tag="stq")
                        nc.vector.bn_stats(stq[:], p_qc[:])
                        mvq = sp.tile([128, 2], f32, tag="mvq")
                        nc.vector.bn_aggr(mvq[:], stq[:])
                        rsq = sp.tile([128, 1], f32, tag="rsq")
                        nc.scalar.activation(rsq[:], mvq[:, 1:2], AF.Sqrt,
                                             bias=eps_t[:])
                        rsq2 = sp.tile([128, 1], f32, tag="rsq2")
                        nc.vector.reciprocal(rsq2[:], rsq[:])
                        nmq = sp.tile([128, 1], f32, tag="nmq")
                        nc.vector.tensor_scalar(
                            out=nmq[:], in0=mvq[:, 0:1], scalar1=rsq2[:],
                            scalar2=-1.0, op0=ALU.mult, op1=ALU.mult)
                        cn_t = sp.tile([128, 1024], bf16, tag="cn", bufs=4)
                        nc.vector.tensor_scalar(
                            out=cn_t[:, 0:512], in0=p_qc[:], scalar1=rsq2[:],
                            scalar2=nmq[:], op0=ALU.mult, op1=ALU.add)
                        # LN on ckv (272 + 240 chunks)
                        stk = sp.tile([128, 2, 6], f32, tag="stk")
                        nc.vector.bn_stats(stk[:, 0, :], p_ka[:])
                        nc.vector.bn_stats(stk[:, 1, :], p_kb[:, 0:240])
                        mvk = sp.tile([128, 2], f32, tag="mvk")
                        nc.vector.bn_aggr(mvk[:], stk[:])
                        rsk = sp.tile([128, 1], f32, tag="rsk")
                        nc.scalar.activation(rsk[:], mvk[:, 1:2], AF.Sqrt,
                                             bias=eps_t[:])
                        rsk2 = sp.tile([128, 1], f32, tag="rsk2")
                        nc.vector.reciprocal(rsk2[:], rsk[:])
                        nmk = sp.tile([128, 1], f32, tag="nmk")
                        nc.vector.tensor_scalar(
                            out=nmk[:], in0=mvk[:, 0:1], scalar1=rsk2[:],
                            scalar2=-1.0, op0=ALU.mult, op1=ALU.mult)
                        nc.scalar.activation(cn_t[:, 512:784], p_ka[:],
                                             AF.Identity, bias=nmk[:],
                                             scale=rsk2[:])
                        nc.vector.tensor_scalar(
                            out=cn_t[:, 784:1024], in0=p_kb[:, 0:240],
                            scalar1=rsk2[:], scalar2=nmk[:],
                            op0=ALU.mult, op1=ALU.add)
                        # kpe rope (raw cols 240:272 of p_kb)
                        kraw = sp.tile([128, ROPE], bf16, tag="kraw")
                        nc.vector.tensor_copy(kraw[:], p_kb[:, 240:272])
                        ksw = sp.tile([128, ROPE], bf16, tag="ksw")
                        kraw3 = kraw.rearrange("p (i two) -> p i two", two=2)
                        ksw3 = ksw.rearrange("p (i two) -> p i two", two=2)
                        nc.gpsimd.tensor_copy(ksw3[:, :, 0:1], kraw3[:, :, 1:2])
                        nc.gpsimd.tensor_copy(ksw3[:, :, 1:2], kraw3[:, :, 0:1])
                        c2t = c2k_t[:, t, :]
                        s2t = s2k_t[:, t, :]
                        kp1 = sp.tile([128, ROPE], bf16, tag="kp1")
                        nc.gpsimd.tensor_mul(kp1[:], kraw[:], c2t[:])
                        kp2 = sp.tile([128, ROPE], bf16, tag="kp2")
                        nc.gpsimd.tensor_mul(kp2[:], ksw[:], s2t[:])
                        nc.gpsimd.tensor_add(kpe_all[:, t, :], kp1[:], kp2[:])
                        # feature-major via DMA XBAR transpose
                        nc.sync.dma_start_transpose(cnT[:, :, ts_], cn_t[:])

                # ---------------- stage 2: qb / kvb + rope + assemble -----
                acts2 = top.enter_context(tc.tile_pool(name="acts2", bufs=1))
                # j 0-1: q head-pairs; j 2-3: k head-pairs
                qkfT = acts2.tile([128, 4, S], bf16, tag="qkfT")
                kvd = acts2.tile([128, TT, KVW], bf16, tag="kvd")
                wqb_t = wq2.tile([128, RT, DPC], bf16, tag="wqb")
                nc.scalar.dma_start(
                    wqb_t[:], wqb_d.rearrange("(k p) n -> p k n", p=128))
                wkvb_t = wq2.tile([128, RT, KVW], bf16, tag="wkvb")
                nc.scalar.dma_start(
                    wkvb_t[:], wkvb_d.rearrange("(k p) n -> p k n", p=128))
                wout_t = wq2.tile([128, 2, E], bf16, tag="wout")
                nc.scalar.dma_start(
                    wout_t[:], wout_d.rearrange("(k p) n -> p k n", p=128))

                with contextlib.ExitStack() as st2:
                    ab2 = st2.enter_context(tc.tile_pool(name="ab2", bufs=1))
                    aq_t = ab2.tile([128, TT, DPC], bf16, tag="aq")
                    nc.scalar.dma_start(
                        aq_t[:], aq_d.rearrange("(t p) d -> p t d", p=128))
                    bq_t = ab2.tile([128, TT, DPC], bf16, tag="bq")
                    nc.scalar.dma_start(
                        bq_t[:], bq_d.rearrange("(t p) d -> p t d", p=128))
                    pq = st2.enter_context(
                        tc.tile_pool(name="pq", bufs=2, space="PSUM"))
                    pkv = st2.enter_context(
                        tc.tile_pool(name="pkv", bufs=2, space="PSUM"))
                    qk2 = st2.enter_context(tc.tile_pool(name="qk2", bufs=2))

                    for t in range(TT):
                        ts_ = slice(t * 128, (t + 1) * 128)
                        qkf = qk2.tile([128, 512], bf16, tag="qkf", bufs=3)
                        # qb
                        p_q = pq.tile([128, DPC], f32, tag="p_q")
                        for r in range(RT):
                            nc.tensor.matmul(p_q[:], cnT[:, r, ts_],
                                             wqb_t[:, r, :],
                                             start=(r == 0), stop=False)
                        nc.tensor.matmul(p_q[:], ones1_t[:], qconst_t[:],
                                         start=False, stop=True)
                        q_t = qk2.tile([128, DPC], bf16, tag="q_t")
                        nc.scalar.activation(q_t[:], p_q[:], AF.Copy)
                        # rope on q
                        q_sw = qk2.tile([128, DPC], bf16, tag="q_sw")
                        q3 = q_t.rearrange("p (i two) -> p i two", two=2)
                        qs3 = q_sw.rearrange("p (i two) -> p i two", two=2)
                        nc.gpsimd.tensor_copy(qs3[:, :, 0:1], q3[:, :, 1:2])
                        nc.gpsimd.tensor_copy(qs3[:, :, 1:2], q3[:, :, 0:1])
                        qt1 = qk2.tile([128, DPC], bf16, tag="qt1")
                        nc.vector.tensor_mul(qt1[:], q_t[:], aq_t[:, t, :])
                        qt2 = qk2.tile([128, DPC], bf16, tag="qt2")
                        nc.vector.tensor_mul(qt2[:], q_sw[:], bq_t[:, t, :])
                        nc.vector.tensor_add(qkf[:, 0:DPC], qt1[:], qt2[:])
                        # kvb
                        p_kv = pkv.tile([128, KVW], f32, tag="p_kv")
                        for r in range(RT):
                            nc.tensor.matmul(p_kv[:], cnT[:, RT + r, ts_],
                                             wkvb_t[:, r, :],
                                             start=(r == 0), stop=False)
                        nc.tensor.matmul(p_kv[:], ones1_t[:], kconst_t[:],
                                         start=False, stop=True)
                        nc.vector.tensor_copy(kvd[:, t, :], p_kv[:])
                        kvd3 = kvd.rearrange("p u (h c) -> p u h c", h=HPC)
                        nc.gpsimd.tensor_copy(
                            kvd3[:, t, :, 96:97],
                            ones4_t.rearrange("p (h o) -> p h o", o=1)[:])
                        # k_full assembly into qkf[:, 256:512]
                        kf3 = qkf.rearrange("p (g c) -> p g c", g=8)
                        for h in range(HPC):
                            nc.gpsimd.tensor_copy(kf3[:, 4 + h, 0:32],
                                                  kvd3[:, t, h, 0:32])
                            nc.gpsimd.tensor_copy(kf3[:, 4 + h, 32:64],
                                                  kpe_all[:, t, :])
                        nc.scalar.dma_start_transpose(qkfT[:, :, ts_], qkf[:])

                # ---------------- stage 3+4: attention + out projection ---
                with contextlib.ExitStack() as st3:
                    ps_s = st3.enter_context(
                        tc.tile_pool(name="ps_s", bufs=2, space="PSUM"))
                    ps_av = st3.enter_context(
                        tc.tile_pool(name="ps_av", bufs=2, space="PSUM"))
                    ps_o = st3.enter_context(
                        tc.tile_pool(name="ps_o", bufs=2, space="PSUM"))
                    ex = st3.enter_context(tc.tile_pool(name="ex", bufs=3))
                    on = st3.enter_context(tc.tile_pool(name="on", bufs=2))
                    ozs = st3.enter_context(tc.tile_pool(name="ozs", bufs=3))
                    osb = st3.enter_context(tc.tile_pool(name="osb", bufs=2))

                    pend3 = []

                    def do_outproj(sb_i, onorm):
                        for tc_i in range(4):
                            tcs = slice(tc_i * 128, (tc_i + 1) * 128)
                            o_t = osb.tile([128, E], f32, tag="o_t",
                                           name=f"o_t_{sb_i}_{tc_i}")
                            for ei in range(2):
                                es = slice(ei * 512, (ei + 1) * 512)
                                p_o = ps_o.tile([128, 512], f32, tag="p_o",
                                                name=f"p_o_{sb_i}_{tc_i}_{ei}")
                                for kk in range(2):
                                    nc.tensor.matmul(
                                        p_o[:], onorm[kk][:, tcs],
                                        wout_t[:, kk, es],
                                        start=(kk == 0), stop=(kk == 1))
                                nc.vector.tensor_copy(o_t[:, es], p_o[:])
                            nc.sync.dma_start(
                                out_d[sb_i * 512 + tc_i * 128:
                                      sb_i * 512 + tc_i * 128 + 128, :],
                                o_t[:])

                    for sb_i in range(SB):
                        ss = slice(sb_i * 512, (sb_i + 1) * 512)
                        onorm = [on.tile([128, 512], bf16, tag=f"on{j}",
                                         name=f"on{j}_{sb_i}")
                                 for j in range(2)]
                        for h in range(HPC):
                            j, half = h // 2, (h % 2) * 64
                            hs = slice(half, half + 64)
                            p_av = ps_av.tile([128, 512], f32, tag="p_av")
                            for u2 in range(UT // 2):
                                p_s = ps_s.tile([128, 1024], f32, tag="p_s")
                                for k2 in range(2):
                                    u = 2 * u2 + k2
                                    us = slice(u * 128, (u + 1) * 128)
                                    nc.tensor.matmul(
                                        p_s[:, k2 * 512:(k2 + 1) * 512],
                                        qkfT[hs, 2 + j, us],
                                        qkfT[hs, j, ss],
                                        start=True, stop=True)
                                e_t = ex.tile([128, 1024], bf16, tag="e_t")
                                nc.scalar.activation(e_t[:], p_s[:], AF.Exp,
                                                     scale=0.125)
                                for k2 in range(2):
                                    u = 2 * u2 + k2
                                    nc.tensor.matmul(
                                        p_av[0:65, :],
                                        kvd[:, u, h * 97 + 32:h * 97 + 97],
                                        e_t[:, k2 * 512:(k2 + 1) * 512],
                                        start=(u == 0), stop=(u == UT - 1))
                            rz = ozs.tile([1, 512], f32, tag="rz")
                            with nc.allow_low_precision(reason="bf16 attn"):
                                nc.vector.reciprocal(rz[:], p_av[64:65, :])
                                zb = ozs.tile([64, 512], f32, tag="zb")
                                nc.gpsimd.partition_broadcast(zb[:], rz[:],
                                                              channels=64)
                                nc.vector.tensor_mul(onorm[h // 2][hs, :],
                                                     p_av[0:64, :], zb[:])
                        pend3.append((sb_i, onorm))
                        if len(pend3) > 1:
                            do_outproj(*pend3.pop(0))
                    for args in pend3:
                        do_outproj(*args)

            if reps == 1:
                body()
            else:
                with tc.For_i(0, reps, 1):
                    body()

    nc.compile()
    return nc


def _host_prep(x, Wqa, g_qa, b_qa, Wqb, Wkva, g_kva, b_kva, Wkvb, Wout):
    import ml_dtypes
    f32 = np.float32
    bf16 = ml_dtypes.bfloat16
    x = np.asarray(x, f32)
    Wqa = np.asarray(Wqa, f32); Wqb = np.asarray(Wqb, f32)
    Wkva = np.asarray(Wkva, f32); Wkvb = np.asarray(Wkvb, f32)
    Wout = np.asarray(Wout, f32)
    g_qa = np.asarray(g_qa, f32); b_qa = np.asarray(b_qa, f32)
    g_kva = np.asarray(g_kva, f32); b_kva = np.asarray(b_kva, f32)

    inv = 1.0 / (10000.0 ** (np.arange(0, ROPE, 2, dtype=f32) / ROPE))
    fr = np.arange(S, dtype=f32)[:, None] * inv[None, :]
    cos, sin = np.cos(fr).astype(f32), np.sin(fr).astype(f32)
    c2 = np.repeat(cos, 2, axis=1)
    s2 = np.empty((S, ROPE), f32)
    s2[:, 0::2] = -sin
    s2[:, 1::2] = sin
    Aq = np.ones((S, DPC), f32)
    Bq = np.zeros((S, DPC), f32)
    for h in range(HPC):
        Aq[:, h * 64 + 32:h * 64 + 64] = c2
        Bq[:, h * 64 + 32:h * 64 + 64] = s2

    shared = {
        "WqaT": np.ascontiguousarray(Wqa.T).astype(bf16),
        "WkvaT": np.ascontiguousarray(Wkva.T).astype(bf16),
        "Aq": Aq.astype(bf16), "Bq": Bq.astype(bf16),
        "c2k": c2.astype(bf16), "s2k": s2.astype(bf16),
        "ones1": np.ones((1, 128), bf16),
        "ones4": np.ones((128, 4), bf16),
        "epst": np.full((128, 1), EPS, f32),
    }
    in_maps = []
    for core in range(NCORES):
        b, hg = core // HPC, core % HPC
        Wqb_sl = Wqb[hg * DPC:(hg + 1) * DPC, :]
        WkvbT_eff = np.zeros((KVL, KVW), f32)
        kconst = np.zeros((1, KVW), f32)
        for h in range(HPC):
            blk = Wkvb[(hg * HPC + h) * 96:(hg * HPC + h + 1) * 96, :] \
                * g_kva[None, :]
            WkvbT_eff[:, h * 97:h * 97 + 96] = blk.T
            kconst[0, h * 97:h * 97 + 96] = b_kva @ blk.T
        m = dict(shared)
        m["xT"] = np.ascontiguousarray(x[b].T).astype(bf16)
        m["WqbT"] = np.ascontiguousarray(
            (Wqb_sl * g_qa[None, :]).T).astype(bf16)
        m["qconst"] = (b_qa @ Wqb_sl.T)[None, :].astype(bf16)
        m["WkvbT"] = WkvbT_eff.astype(bf16)
        m["kconst"] = kconst.astype(bf16)
        m["WoutT"] = np.ascontiguousarray(
            Wout[:, hg * DPC:(hg + 1) * DPC].T).astype(bf16)
        in_maps.append(m)
    return in_maps


def kernel(**inputs):
    from concourse.bass_utils import run_bass_kernel_spmd
    if "nc" not in _CACHE:
        _CACHE["nc"] = _build(reps=1)
    nc = _CACHE["nc"]
    in_maps = _host_prep(**inputs)
    res = run_bass_kernel_spmd(nc, in_maps, core_ids=list(range(NCORES)))
    out = np.zeros((B, S, E), np.float32)
    for core in range(NCORES):
        out[core // HPC] += res.results[core]["out"]
    return out


# revision 27
# speedup vs baseline: 1.4885x; 1.4885x over previous
"""MLA encoder self-attention on 8 TRN2 NeuronCores.

Sharding: data-parallel over batch (B=2) x tensor-parallel over head groups
(16 heads -> 4 groups of 4). Core c handles batch c//4, heads 4*(c%4)..+4.
Each core computes its heads' attention and a partial output projection;
the host sums the 4 head-group partials per batch.

Numerics: activations/weights are bf16 (inputs to every matmul); PSUM
accumulation is fp32; LayerNorm statistics are computed in fp32 from the
fp32 PSUM tiles. End-to-end rel err vs the f32 reference ~2e-3, well under
the 2e-2 gate.

Layout notes:
- Activations flow token-major (tokens on partitions) for LayerNorm/rope.
  Feature-major copies for the next contraction are produced by the DMA
  XBAR transpose (2-byte dtype), not the PE: stage1 emits one combined
  [128tok, 1024] tile (qcn|ckvn) per token tile and transposes it into
  cnT[:, r, :] (r 0-3 = qcn feature rows, 4-7 = ckvn); stage2 emits one
  combined [128tok, 512] tile (qf|kf) transposed into qkfT[:, j, :]
  (j 0-1 = q head-pairs, 2-3 = k head-pairs).
- kvd layout per head: [nope(32) | v(64) | ones(1)] stride 97, so v_aug
  ([v | 1]) is a contiguous 65-column slice: the AV matmul computes both
  the attention numerator and the softmax denominator (ones row) at once.
- Softmax has no max-subtraction (scores are bounded ~2, verified on host
  data) and normalization is deferred: out_aug^T rows 0..63 are scaled by
  1/Z (row 64) right before the output projection.
- Scores for two key tiles accumulate into one [128,1024] 2-bank PSUM
  tile so exp runs as half as many, twice as wide Act instructions.
"""
import numpy as np

B, S, E = 2, 2048, 1024
H, HD = 16, 64
ROPE, NOPE = 32, 32
QL, KVL = 512, 512
EPS = 1e-5
HPC = 4                # heads per core
DPC = HPC * HD         # 256
NCORES = 8
TT = S // 128          # 16 token tiles
ET = E // 128          # 8
RT = QL // 128         # 4
SB = S // 512          # 4 s-blocks
UT = S // 128          # 16 key tiles
KVW = 4 * 97           # 388: kvd width

_CACHE = {}


def _build(reps=1):
    import concourse.tile as tile
    import concourse.mybir as mybir
    from concourse import bacc

    f32 = mybir.dt.float32
    bf16 = mybir.dt.bfloat16
    AF = mybir.ActivationFunctionType
    ALU = mybir.AluOpType

    nc = bacc.Bacc("TRN2", target_bir_lowering=False, debug=False,
                   num_devices=NCORES)

    def din(name, shape, dt=bf16):
        return nc.dram_tensor(name, shape, dt, kind="ExternalInput").ap()

    xT_d = din("xT", (E, S))
    wqa_d = din("WqaT", (E, QL))
    wkva_d = din("WkvaT", (E, KVL + ROPE))
    wqb_d = din("WqbT", (QL, DPC))
    qconst_d = din("qconst", (1, DPC))
    wkvb_d = din("WkvbT", (KVL, KVW))
    kconst_d = din("kconst", (1, KVW))
    wout_d = din("WoutT", (DPC, E))
    aq_d = din("Aq", (S, DPC))
    bq_d = din("Bq", (S, DPC))
    c2k_d = din("c2k", (S, ROPE))
    s2k_d = din("s2k", (S, ROPE))
    ones1_d = din("ones1", (1, 128))
    ones4_d = din("ones4", (128, 4))
    eps_d = din("epst", (128, 1), mybir.dt.float32)
    out_d = nc.dram_tensor("out", (S, E), f32, kind="ExternalOutput").ap()

    with tile.TileContext(nc) as tc:
        import contextlib
        with contextlib.ExitStack() as top:
            consts = top.enter_context(tc.tile_pool(name="consts", bufs=1))
            ones1_t = consts.tile([1, 128], bf16, tag="ones1")
            nc.sync.dma_start(ones1_t[:], ones1_d[:])
            ones4_t = consts.tile([128, 4], bf16, tag="ones4")
            nc.sync.dma_start(ones4_t[:], ones4_d[:])
            qconst_t = consts.tile([1, DPC], bf16, tag="qconst")
            nc.sync.dma_start(qconst_t[:], qconst_d[:])
            kconst_t = consts.tile([1, KVW], bf16, tag="kconst")
            nc.sync.dma_start(kconst_t[:], kconst_d[:])
            eps_t = consts.tile([128, 1], f32, tag="epst")
            nc.sync.dma_start(eps_t[:], eps_d[:])

            acts = top.enter_context(tc.tile_pool(name="acts", bufs=1))
            wq2 = top.enter_context(tc.tile_pool(name="wq2", bufs=1))

            def body():
                # feature-major LN'd activations: r 0-3 qcn, 4-7 ckvn
                cnT = acts.tile([128, 2 * RT, S], bf16, tag="cnT")
                kpe_all = acts.tile([128, TT, ROPE], bf16, tag="kpe")

                # ---------------- stage 1: qa + kva + LN + kpe rope -------
                with contextlib.ExitStack() as st1:
                    wqa_p = st1.enter_context(tc.tile_pool(name="wqa", bufs=1))
                    wqa_t = wqa_p.tile([128, ET, QL], bf16, tag="wqa")
                    wkva_t = wqa_p.tile([128, ET, KVL + ROPE], bf16,
                                        tag="wkva")

                    pmm = st1.enter_context(
                        tc.tile_pool(name="pmm", bufs=2, space="PSUM"))
                    xp = st1.enter_context(tc.tile_pool(name="xp", bufs=3))
                    sp = st1.enter_context(tc.tile_pool(name="sp", bufs=3))
                    ck = st1.enter_context(tc.tile_pool(name="ck", bufs=1))
                    c2k_t = ck.tile([128, TT, ROPE], bf16, tag="c2k")
                    nc.gpsimd.dma_start(
                        c2k_t[:], c2k_d.rearrange("(t p) r -> p t r", p=128))
                    s2k_t = ck.tile([128, TT, ROPE], bf16, tag="s2k")
                    nc.gpsimd.dma_start(
                        s2k_t[:], s2k_d.rearrange("(t p) r -> p t r", p=128))

                    for t in range(TT):
                        ts_ = slice(t * 128, (t + 1) * 128)
                        p_qc = pmm.tile([128, QL], f32, tag="p_qc")
                        p_ka = pmm.tile([128, 272], f32, tag="p_ka")
                        p_kb = pmm.tile([128, 272], f32, tag="p_kb")
                        xt = xp.tile([128, ET, 128], bf16, tag="xt")
                        if t == 0:
                            for e in range(ET):
                                es_ = slice(e * 128, (e + 1) * 128)
                                nc.sync.dma_start(
                                    xt[:, e, :], xT_d[es_, ts_])
                                nc.scalar.dma_start(wqa_t[:, e, :],
                                                    wqa_d[es_, :])
                                nc.scalar.dma_start(wkva_t[:, e, :],
                                                    wkva_d[es_, :])
                        else:
                            nc.sync.dma_start(
                                xt[:],
                                xT_d.rearrange("(k p) s -> p k s", p=128)[:, :, ts_])
                        for e in range(ET):
                            st, sp_ = (e == 0), (e == ET - 1)
                            nc.tensor.matmul(p_qc[:], xt[:, e, :], wqa_t[:, e, :],
                                             start=st, stop=sp_)
                            nc.tensor.matmul(p_ka[:], xt[:, e, :],
                                             wkva_t[:, e, 0:272],
                                             start=st, stop=sp_)
                            nc.tensor.matmul(p_kb[:], xt[:, e, :],
                                             wkva_t[:, e, 272:544],
                                             start=st, stop=sp_)
                        # LN on qc
                        stq = sp.tile([128, 6], f32, tag="stq")
                        nc.vector.bn_stats(stq[:], p_qc[:])
                        mvq = sp.tile([128, 2], f32, tag="mvq")
                        nc.vector.bn_aggr(mvq[:], stq[:])
                        rsq = sp.tile([128, 1], f32, tag="rsq")
                        nc.scalar.activation(rsq[:], mvq[:, 1:2], AF.Sqrt,
                                             bias=eps_t[:])
                        rsq2 = sp.tile([128, 1], f32, tag="rsq2")
                        nc.vector.reciprocal(rsq2[:], rsq[:])
                        nmq = sp.tile([128, 1], f32, tag="nmq")
                        nc.vector.tensor_scalar(
                            out=nmq[:], in0=mvq[:, 0:1], scalar1=rsq2[:],
                            scalar2=-1.0, op0=ALU.mult, op1=ALU.mult)
                        cn_t = sp.tile([128, 1024], bf16, tag="cn", bufs=4)
                        nc.vector.tensor_scalar(
                            out=cn_t[:, 0:512], in0=p_qc[:], scalar1=rsq2[:],
                            scalar2=nmq[:], op0=ALU.mult, op1=ALU.add)
                        # LN on ckv (272 + 240 chunks)
                        stk = sp.tile([128, 2, 6], f32, tag="stk")
                        nc.vector.bn_stats(stk[:, 0, :], p_ka[:])
                        nc.vector.bn_stats(stk[:, 1, :], p_kb[:, 0:240])
                        mvk = sp.tile([128, 2], f32, tag="mvk")
                        nc.vector.bn_aggr(mvk[:], stk[:])
                        rsk = sp.tile([128, 1], f32, tag="rsk")
                        nc.scalar.activation(rsk[:], mvk[:, 1:2], AF.Sqrt,
                                             bias=eps_t[:])
                        rsk2 = sp.tile([128, 1], f32, tag="rsk2")
                        nc.vector.reciprocal(rsk2[:], rsk[:])
                        nmk = sp.tile([128, 1], f32, tag="nmk")
                        nc.vector.tensor_scalar(
                            out=nmk[:], in0=mvk[:, 0:1], scalar1=rsk2[:],
                            scalar2=-1.0, op0=ALU.mult, op1=ALU.mult)
                        nc.scalar.activation(cn_t[:, 512:784], p_ka[:],
                                             AF.Identity, bias=nmk[:],
                                             scale=rsk2[:])
                        nc.vector.tensor_scalar(
                            out=cn_t[:, 784:1024], in0=p_kb[:, 0:240],
                            scalar1=rsk2[:], scalar2=nmk[:],
                            op0=ALU.mult, op1=ALU.add)
                        # kpe rope (raw cols 240:272 of p_kb)
                        kraw = sp.tile([128, ROPE], bf16, tag="kraw")
                        nc.vector.tensor_copy(kraw[:], p_kb[:, 240:272])
                        ksw = sp.tile([128, ROPE], bf16, tag="ksw")
                        kraw3 = kraw.rearrange("p (i two) -> p i two", two=2)
                        ksw3 = ksw.rearrange("p (i two) -> p i two", two=2)
                        nc.gpsimd.tensor_copy(ksw3[:, :, 0:1], kraw3[:, :, 1:2])
                        nc.gpsimd.tensor_copy(ksw3[:, :, 1:2], kraw3[:, :, 0:1])
                        c2t = c2k_t[:, t, :]
                        s2t = s2k_t[:, t, :]
                        kp1 = sp.tile([128, ROPE], bf16, tag="kp1")
                        nc.gpsimd.tensor_mul(kp1[:], kraw[:], c2t[:])
                        kp2 = sp.tile([128, ROPE], bf16, tag="kp2")
                        nc.gpsimd.tensor_mul(kp2[:], ksw[:], s2t[:])
                        nc.gpsimd.tensor_add(kpe_all[:, t, :], kp1[:], kp2[:])
                        # feature-major via DMA XBAR transpose
                        nc.sync.dma_start_transpose(cnT[:, :, ts_], cn_t[:])

                # ---------------- stage 2: qb / kvb + rope + assemble -----
                acts2 = top.enter_context(tc.tile_pool(name="acts2", bufs=1))
                # slot g (0-3): q head g in partitions 0-63, zeros 64-127;
                # slot 4+h: k head h in partitions 0-63, zeros 64-127.
                # Zero-padding to K=128 makes the score matmuls run at the
                # full-partition rate (K=64 matmuls measured 1.6x slower).
                qkfT = acts2.tile([128, 8, S], bf16, tag="qkfT")
                kvd = acts2.tile([128, TT, KVW], bf16, tag="kvd")
                wqb_t = wq2.tile([128, RT, DPC], bf16, tag="wqb")
                nc.scalar.dma_start(
                    wqb_t[:], wqb_d.rearrange("(k p) n -> p k n", p=128))
                wkvb_t = wq2.tile([128, RT, KVW], bf16, tag="wkvb")
                nc.scalar.dma_start(
                    wkvb_t[:], wkvb_d.rearrange("(k p) n -> p k n", p=128))
                wout_t = wq2.tile([128, 2, E], bf16, tag="wout")
                nc.scalar.dma_start(
                    wout_t[:], wout_d.rearrange("(k p) n -> p k n", p=128))

                with contextlib.ExitStack() as st2:
                    ab2 = st2.enter_context(tc.tile_pool(name="ab2", bufs=1))
                    aq_t = ab2.tile([128, TT, DPC], bf16, tag="aq")
                    nc.scalar.dma_start(
                        aq_t[:], aq_d.rearrange("(t p) d -> p t d", p=128))
                    bq_t = ab2.tile([128, TT, DPC], bf16, tag="bq")
                    nc.scalar.dma_start(
                        bq_t[:], bq_d.rearrange("(t p) d -> p t d", p=128))
                    pq = st2.enter_context(
                        tc.tile_pool(name="pq", bufs=2, space="PSUM"))
                    pkv = st2.enter_context(
                        tc.tile_pool(name="pkv", bufs=2, space="PSUM"))
                    qk2 = st2.enter_context(tc.tile_pool(name="qk2", bufs=2))

                    for t in range(TT):
                        ts_ = slice(t * 128, (t + 1) * 128)
                        qkf = qk2.tile([128, 1024], bf16, tag="qkf", bufs=3)
                        qkf_g = qkf.rearrange("p (g c) -> p g c", c=128)
                        # zero the pad halves of all 8 slots
                        nc.gpsimd.memset(qkf_g[:, :, 64:128], 0.0)
                        # qb
                        p_q = pq.tile([128, DPC], f32, tag="p_q")
                        for r in range(RT):
                            nc.tensor.matmul(p_q[:], cnT[:, r, ts_],
                                             wqb_t[:, r, :],
                                             start=(r == 0), stop=False)
                        nc.tensor.matmul(p_q[:], ones1_t[:], qconst_t[:],
                                         start=False, stop=True)
                        q_t = qk2.tile([128, DPC], bf16, tag="q_t")
                        nc.scalar.activation(q_t[:], p_q[:], AF.Copy)
                        # rope on q
                        q_sw = qk2.tile([128, DPC], bf16, tag="q_sw")
                        q3 = q_t.rearrange("p (i two) -> p i two", two=2)
                        qs3 = q_sw.rearrange("p (i two) -> p i two", two=2)
                        nc.gpsimd.tensor_copy(qs3[:, :, 0:1], q3[:, :, 1:2])
                        nc.gpsimd.tensor_copy(qs3[:, :, 1:2], q3[:, :, 0:1])
                        qt1 = qk2.tile([128, DPC], bf16, tag="qt1")
                        nc.vector.tensor_mul(qt1[:], q_t[:], aq_t[:, t, :])
                        qt2 = qk2.tile([128, DPC], bf16, tag="qt2")
                        nc.vector.tensor_mul(qt2[:], q_sw[:], bq_t[:, t, :])
                        nc.vector.tensor_add(
                            qkf_g[:, 0:4, 0:64],
                            qt1.rearrange("p (h c) -> p h c", c=64)[:],
                            qt2.rearrange("p (h c) -> p h c", c=64)[:])
                        # kvb
                        p_kv = pkv.tile([128, KVW], f32, tag="p_kv")
                        for r in range(RT):
                            nc.tensor.matmul(p_kv[:], cnT[:, RT + r, ts_],
                                             wkvb_t[:, r, :],
                                             start=(r == 0), stop=False)
                        nc.tensor.matmul(p_kv[:], ones1_t[:], kconst_t[:],
                                         start=False, stop=True)
                        nc.vector.tensor_copy(kvd[:, t, :], p_kv[:])
                        kvd3 = kvd.rearrange("p u (h c) -> p u h c", h=HPC)
                        nc.gpsimd.tensor_copy(
                            kvd3[:, t, :, 96:97],
                            ones4_t.rearrange("p (h o) -> p h o", o=1)[:])
                        # k_full assembly into slots 4-7
                        for h in range(HPC):
                            nc.gpsimd.tensor_copy(qkf_g[:, 4 + h, 0:32],
                                                  kvd3[:, t, h, 0:32])
                            nc.gpsimd.tensor_copy(qkf_g[:, 4 + h, 32:64],
                                                  kpe_all[:, t, :])
                        nc.scalar.dma_start_transpose(qkfT[:, :, ts_], qkf[:])

                # ---------------- stage 3+4: attention + out projection ---
                with contextlib.ExitStack() as st3:
                    ps_s = st3.enter_context(
                        tc.tile_pool(name="ps_s", bufs=2, space="PSUM"))
                    ps_av = st3.enter_context(
                        tc.tile_pool(name="ps_av", bufs=2, space="PSUM"))
                    ps_o = st3.enter_context(
                        tc.tile_pool(name="ps_o", bufs=2, space="PSUM"))
                    ex = st3.enter_context(tc.tile_pool(name="ex", bufs=3))
                    on = st3.enter_context(tc.tile_pool(name="on", bufs=2))
                    ozs = st3.enter_context(tc.tile_pool(name="ozs", bufs=3))
                    osb = st3.enter_context(tc.tile_pool(name="osb", bufs=2))

                    pend3 = []

                    def do_outproj(sb_i, onorm):
                        for tc_i in range(4):
                            tcs = slice(tc_i * 128, (tc_i + 1) * 128)
                            o_t = osb.tile([128, E], f32, tag="o_t",
                                           name=f"o_t_{sb_i}_{tc_i}")
                            for ei in range(2):
                                es = slice(ei * 512, (ei + 1) * 512)
                                p_o = ps_o.tile([128, 512], f32, tag="p_o",
                                                name=f"p_o_{sb_i}_{tc_i}_{ei}")
                                for kk in range(2):
                                    nc.tensor.matmul(
                                        p_o[:], onorm[kk][:, tcs],
                                        wout_t[:, kk, es],
                                        start=(kk == 0), stop=(kk == 1))
                                nc.vector.tensor_copy(o_t[:, es], p_o[:])
                            nc.sync.dma_start(
                                out_d[sb_i * 512 + tc_i * 128:
                                      sb_i * 512 + tc_i * 128 + 128, :],
                                o_t[:])

                    for sb_i in range(SB):
                        ss = slice(sb_i * 512, (sb_i + 1) * 512)
                        onorm = [on.tile([128, 512], bf16, tag=f"on{j}",
                                         name=f"on{j}_{sb_i}")
                                 for j in range(2)]
                        for h in range(HPC):
                            hs = slice((h % 2) * 64, (h % 2) * 64 + 64)
                            p_av = ps_av.tile([128, 512], f32, tag="p_av")
                            for u2 in range(UT // 2):
                                p_s = ps_s.tile([128, 1024], f32, tag="p_s")
                                for k2 in range(2):
                                    u = 2 * u2 + k2
                                    us = slice(u * 128, (u + 1) * 128)
                                    nc.tensor.matmul(
                                        p_s[:, k2 * 512:(k2 + 1) * 512],
                                        qkfT[:, 4 + h, us],
                                        qkfT[:, h, ss],
                                        start=True, stop=True)
                                e_t = ex.tile([128, 1024], bf16, tag="e_t")
                                nc.scalar.activation(e_t[:], p_s[:], AF.Exp,
                                                     scale=0.125)
                                for k2 in range(2):
                                    u = 2 * u2 + k2
                                    nc.tensor.matmul(
                                        p_av[0:65, :],
                                        kvd[:, u, h * 97 + 32:h * 97 + 97],
                                        e_t[:, k2 * 512:(k2 + 1) * 512],
                                        start=(u == 0), stop=(u == UT - 1))
                            rz = ozs.tile([1, 512], f32, tag="rz")
                            with nc.allow_low_precision(reason="bf16 attn"):
                                nc.vector.reciprocal(rz[:], p_av[64:65, :])
                                zb = ozs.tile([64, 512], f32, tag="zb")
                                nc.gpsimd.partition_broadcast(zb[:], rz[:],
                                                              channels=64)
                                nc.vector.tensor_mul(onorm[h // 2][hs, :],
                                                     p_av[0:64, :], zb[:])
                        pend3.append((sb_i, onorm))
                        if len(pend3) > 1:
                            do_outproj(*pend3.pop(0))
                    for args in pend3:
                        do_outproj(*args)

            if reps == 1:
                body()
            else:
                with tc.For_i(0, reps, 1):
                    body()

    nc.compile()
    return nc


def _host_prep(x, Wqa, g_qa, b_qa, Wqb, Wkva, g_kva, b_kva, Wkvb, Wout):
    import ml_dtypes
    f32 = np.float32
    bf16 = ml_dtypes.bfloat16
    x = np.asarray(x, f32)
    Wqa = np.asarray(Wqa, f32); Wqb = np.asarray(Wqb, f32)
    Wkva = np.asarray(Wkva, f32); Wkvb = np.asarray(Wkvb, f32)
    Wout = np.asarray(Wout, f32)
    g_qa = np.asarray(g_qa, f32); b_qa = np.asarray(b_qa, f32)
    g_kva = np.asarray(g_kva, f32); b_kva = np.asarray(b_kva, f32)

    inv = 1.0 / (10000.0 ** (np.arange(0, ROPE, 2, dtype=f32) / ROPE))
    fr = np.arange(S, dtype=f32)[:, None] * inv[None, :]
    cos, sin = np.cos(fr).astype(f32), np.sin(fr).astype(f32)
    c2 = np.repeat(cos, 2, axis=1)
    s2 = np.empty((S, ROPE), f32)
    s2[:, 0::2] = -sin
    s2[:, 1::2] = sin
    Aq = np.ones((S, DPC), f32)
    Bq = np.zeros((S, DPC), f32)
    for h in range(HPC):
        Aq[:, h * 64 + 32:h * 64 + 64] = c2
        Bq[:, h * 64 + 32:h * 64 + 64] = s2

    shared = {
        "WqaT": np.ascontiguousarray(Wqa.T).astype(bf16),
        "WkvaT": np.ascontiguousarray(Wkva.T).astype(bf16),
        "Aq": Aq.astype(bf16), "Bq": Bq.astype(bf16),
        "c2k": c2.astype(bf16), "s2k": s2.astype(bf16),
        "ones1": np.ones((1, 128), bf16),
        "ones4": np.ones((128, 4), bf16),
        "epst": np.full((128, 1), EPS, f32),
    }
    in_maps = []
    for core in range(NCORES):
        b, hg = core // HPC, core % HPC
        Wqb_sl = Wqb[hg * DPC:(hg + 1) * DPC, :]
        WkvbT_eff = np.zeros((KVL, KVW), f32)
        kconst = np.zeros((1, KVW), f32)
        for h in range(HPC):
            blk = Wkvb[(hg * HPC + h) * 96:(hg * HPC + h + 1) * 96, :] \
                * g_kva[None, :]
            WkvbT_eff[:, h * 97:h * 97 + 96] = blk.T
            kconst[0, h * 97:h * 97 + 96] = b_kva @ blk.T
        m = dict(shared)
        m["xT"] = np.ascontiguousarray(x[b].T).astype(bf16)
        m["WqbT"] = np.ascontiguousarray(
            (Wqb_sl * g_qa[None, :]).T).astype(bf16)
        m["qconst"] = (b_qa @ Wqb_sl.T)[None, :].astype(bf16)
        m["WkvbT"] = WkvbT_eff.astype(bf16)
        m["kconst"] = kconst.astype(bf16)
        m["WoutT"] = np.ascontiguousarray(
            Wout[:, hg * DPC:(hg + 1) * DPC].T).astype(bf16)
        in_maps.append(m)
    return in_maps


def kernel(**inputs):
    from concourse.bass_utils import run_bass_kernel_spmd
    if "nc" not in _CACHE:
        _CACHE["nc"] = _build(reps=1)
    nc = _CACHE["nc"]
    in_maps = _host_prep(**inputs)
    res = run_bass_kernel_spmd(nc, in_maps, core_ids=list(range(NCORES)))
    out = np.zeros((B, S, E), np.float32)
    for core in range(NCORES):
        out[core // HPC] += res.results[core]["out"]
    return out
